# revision 37
# baseline (speedup 1.0000x reference)
"""Trainium2 Bass kernel for nn_DebiasIntraDist (segment_reduce).

Full-input contract: kernel(**inputs) takes the complete (unsharded) inputs
and returns the full scalar loss. The N=65536 samples are sharded across the
8 NeuronCores by (demog, label-half): core 2d+h gets the rows with
demog == d and label-half h (a partition of the N axis). Every core then
owns a disjoint set of 256 (demog, label) groups, so no cross-core
reduction of group accumulators is needed at all - only a tiny final
AllGather of per-core partial (num, den) scalars.

Math: instead of the reference's two-pass (compute mu, then gather mu[seg]
and re-reduce), each core computes per-group
    cnt[g], sums[g, :] (one-hot matmul), sumsq[g] = sum_i ||x_i||^2
and uses the variance decomposition
    sum_{i in g} ||x_i - mu_g||^2 = sumsq[g] - ||sums[g]||^2 / cnt[g]

Matmuls run in bf16 with an exact one-hot stationary operand and an
error-compensated moving operand (X = hi + lo, two passes into the same
PSUM accumulator) - near-fp32 accuracy at bf16 speed (fp32 matmul on TRN2
costs ~4 cycles/row; bf16 costs 1).
"""

import numpy as np

try:
    import concourse.bacc as bacc
except ImportError:  # fresh environment without PYTHONPATH set up
    import sys
    for p in ("/root/.axon_site/_ro/trn_rl_repo", "/opt/trn_rl_repo",
              "/root/.axon_site/_ro/pypackages"):
        if p not in sys.path:
            sys.path.append(p)
    import concourse.bacc as bacc
import concourse.mybir as mybir
import concourse.tile as tile
import concourse.bass_utils as bass_utils

N_CORES = 8
P = 128
D = 512          # feature dim
NL = 256         # labels per core after (demog, label-half) sharding
ND = 4           # demog values
NCH = NL // P    # one-hot chunks of 128 groups
CH = 4           # sample-tiles per feats DMA (1 MiB)

_cache: dict[int, object] = {}


def _build(S: int, debug: bool = False):
    """Compile the SPMD kernel for padded shard length S (multiple of 128)."""
    T = S // P
    fp32 = mybir.dt.float32
    bf16 = mybir.dt.bfloat16
    i32 = mybir.dt.int32
    Alu = mybir.AluOpType
    Act = mybir.ActivationFunctionType

    nc = bacc.Bacc("TRN2", target_bir_lowering=False, debug=False,
                   enable_asserts=True, num_devices=N_CORES)

    feats = nc.dram_tensor("feats", [S, D], fp32, kind="ExternalInput").ap()
    labels_t = nc.dram_tensor("labels_t", [P, T], fp32, kind="ExternalInput").ap()
    loss = nc.dram_tensor("loss", [1, 1], fp32, kind="ExternalOutput").ap()
    if debug:
        dbg_ag = nc.dram_tensor("dbg_ag", [1, 2 * N_CORES], fp32,
                                kind="ExternalOutput").ap()

    with tile.TileContext(nc) as tc:
        with (
            tc.tile_pool(name="const", bufs=1) as constp,
            tc.tile_pool(name="fx", bufs=3) as fxp,
            tc.tile_pool(name="oh", bufs=4) as ohp,
            tc.tile_pool(name="hilo", bufs=3) as hlp,
            tc.tile_pool(name="scr", bufs=3) as scrp,
            tc.tile_pool(name="r3", bufs=4) as r3p,
            tc.tile_pool(name="post", bufs=1) as postp,
            tc.tile_pool(name="ps", bufs=1, space="PSUM") as psp,
            tc.tile_pool(name="dram", bufs=1, space="DRAM") as dram,
        ):
            # constants
            iota = constp.tile([P, NL], i32, tag="iota")
            nc.gpsimd.iota(iota[:], [[1, NL]], channel_multiplier=0)
            labs = constp.tile([P, T], fp32, tag="labs")
            nc.sync.dma_start(out=labs[:], in_=labels_t[:])

            # r3 = [sq_hi, sq_lo, 1] per tile; hand-rotated 4-slot pool so the
            # ones column is written only once, outside the loop
            r3s = []
            for k in range(4):
                r3k = r3p.tile([P, 3], bf16, tag=f"r3_{k}", name=f"r3_{k}")
                nc.gpsimd.memset(r3k[:, 2:3], 1.0)
                r3s.append(r3k)

            # two dummy AllGathers fired at kernel start: they absorb the cold
            # first-collective cost (~40us) concurrently with the main loop,
            # so the real AllGather at the end runs on a warm path
            warm_in = dram.tile([1, 1], fp32)
            warm_out = dram.tile([1, N_CORES], fp32)
            nc.sync.dma_start(out=warm_in[:], in_=labs[:1, :1])
            nc.gpsimd.collective_compute(
                "AllGather", Alu.bypass,
                replica_groups=[[0, 1, 2, 3, 4, 5, 6, 7]],
                ins=[warm_in.opt()], outs=[warm_out.opt()],
            )
            warm_out2 = dram.tile([1, N_CORES], fp32)
            nc.gpsimd.collective_compute(
                "AllGather", Alu.bypass,
                replica_groups=[[0, 1, 2, 3, 4, 5, 6, 7]],
                ins=[warm_in.opt()], outs=[warm_out2.opt()],
            )

            # per-group accumulators (PSUM, accumulated across all T tiles);
            # a PSUM accumulation group owns its whole bank, so each gets one
            ps_sums = [psp.tile([P, D], fp32, tag=f"sums{c}", name=f"sums{c}")
                       for c in range(NCH)]
            ps_small = [psp.tile([P, 3], fp32, tag=f"small{c}", name=f"small{c}")
                        for c in range(NCH)]

            feats_r = feats.rearrange("(n p) d -> n p d", p=P)  # [T, P, D]

            t = 0
            first_chunk = True
            while t < T:
                # small first chunk so PE work starts ASAP
                L = 1 if first_chunk else min(CH, T - t)
                L = min(L, T - t)
                first_chunk = False
                fx = fxp.tile([P, CH, D], fp32, tag="fx")
                nc.sync.dma_start(
                    out=fx[:, :L, :],
                    in_=feats_r[t:t + L].rearrange("n p d -> p n d"),
                )
                fxb = fx[:].bitcast(bf16)  # [P, CH, 2*D] uint16-granular view
                for j in range(L):
                    ti = t + j
                    X = fx[:, j, :]
                    # one-hot of this tile's labels (exact in bf16)
                    oh = ohp.tile([P, NL], bf16, tag="oh")
                    nc.vector.tensor_scalar(
                        out=oh[:], in0=iota[:], scalar1=labs[:, ti:ti + 1],
                        scalar2=None, op0=Alu.is_equal,
                    )
                    # error-compensated bf16 split of X: hi = high 2 bytes of
                    # each fp32 (free truncated-bf16 strided view), lo = X - hi
                    xhi = fxb[:, j, 1::2]
                    xlo = hlp.tile([P, D], bf16, tag="xlo")
                    nc.vector.tensor_tensor(out=xlo[:], in0=X, in1=xhi,
                                            op=Alu.subtract)
                    # row sumsq (fp32) then bf16 hi/lo split + ones column
                    scr = scrp.tile([P, D], fp32, tag="scr")
                    sq = scrp.tile([P, 1], fp32, tag="sq")
                    nc.scalar.activation(scr[:], X, Act.Square,
                                         accum_out=sq[:])
                    r3 = r3s[ti % 4]
                    nc.vector.tensor_copy(out=r3[:, 0:1], in_=sq[:])
                    nc.vector.tensor_tensor(out=r3[:, 1:2], in0=sq[:],
                                            in1=r3[:, 0:1], op=Alu.subtract)

                    st, sp = (ti == 0), (ti == T - 1)
                    for c in range(NCH):
                        ohc = oh[:, c * P:(c + 1) * P]
                        nc.tensor.matmul(out=ps_sums[c][:], lhsT=ohc,
                                         rhs=xhi, start=st, stop=False)
                        nc.tensor.matmul(out=ps_sums[c][:], lhsT=ohc,
                                         rhs=xlo[:], start=False, stop=sp)
                        nc.tensor.matmul(out=ps_small[c][:], lhsT=ohc,
                                         rhs=r3[:], start=st, stop=sp)

                t += L

            # post-processing on this core's 256 groups (no cross-core
            # reduction needed - group sets are disjoint by construction)
            norm2 = postp.tile([P, NCH], fp32, tag="norm2")
            for c in range(NCH):
                scr2 = scrp.tile([P, D], fp32, tag="scr")
                nc.scalar.activation(scr2[:], ps_sums[c][:],
                                     Act.Square, accum_out=norm2[:, c:c + 1])

            small = postp.tile([P, 3 * NCH], fp32, tag="small")
            for c in range(NCH):
                nc.vector.tensor_copy(out=small[:, 3 * c:3 * c + 3],
                                      in_=ps_small[c][:])
            sumsq = postp.tile([P, NCH], fp32, tag="sumsq")
            nc.vector.tensor_tensor(out=sumsq[:], in0=small[:, 0::3],
                                    in1=small[:, 1::3], op=Alu.add)
            cnt = small[:, 2::3]  # [P, NCH]

            safe = postp.tile([P, NCH], fp32, tag="safe")
            nc.vector.tensor_scalar_max(safe[:], cnt, 1.0)
            inv = postp.tile([P, NCH], fp32, tag="inv")
            nc.vector.reciprocal(inv[:], safe[:])
            # grp = (sumsq - norm2 * inv) * inv
            t1 = postp.tile([P, NCH], fp32, tag="t1")
            nc.vector.tensor_tensor(out=t1[:], in0=norm2[:], in1=inv[:],
                                    op=Alu.mult)
            t2 = postp.tile([P, NCH], fp32, tag="t2")
            nc.vector.tensor_tensor(out=t2[:], in0=sumsq[:], in1=t1[:],
                                    op=Alu.subtract)
            grp = postp.tile([P, NCH], fp32, tag="grp")
            nc.vector.tensor_tensor(out=grp[:], in0=t2[:], in1=inv[:],
                                    op=Alu.mult)
            pres = postp.tile([P, NCH], fp32, tag="pres")
            nc.vector.tensor_scalar(out=pres[:], in0=cnt, scalar1=0.0,
                                    scalar2=None, op0=Alu.is_gt)
            # pack [grp*pres | pres]; reduce this core's groups via matmul
            pk = postp.tile([P, 2 * NCH], fp32, tag="pk")
            nc.vector.tensor_tensor(out=pk[:, 0:NCH], in0=grp[:], in1=pres[:],
                                    op=Alu.mult)
            nc.vector.tensor_copy(out=pk[:, NCH:2 * NCH], in_=pres[:])
            ones = constp.tile([P, 1], fp32, tag="ones")
            nc.gpsimd.memset(ones[:], 1.0)
            ps18 = psp.tile([1, 2 * NCH], fp32, tag="small0")
            nc.tensor.matmul(out=ps18[:], lhsT=ones[:], rhs=pk[:],
                             start=True, stop=True)
            s18 = postp.tile([1, 2 * NCH], fp32, tag="s18")
            nc.vector.tensor_copy(out=s18[:], in_=ps18[:])
            # partial num/den over this core's 256 groups
            nd_t = postp.tile([1, 2], fp32, tag="nd_t")
            nc.vector.tensor_reduce(out=nd_t[:, 0:1], in_=s18[:1, 0:NCH],
                                    axis=mybir.AxisListType.X, op=Alu.add)
            nc.vector.tensor_reduce(out=nd_t[:, 1:2], in_=s18[:1, NCH:2 * NCH],
                                    axis=mybir.AxisListType.X, op=Alu.add)

            # gather all 8 cores' (num, den) partials
            ag_in = dram.tile([1, 2], fp32)
            ag_out = dram.tile([1, 2 * N_CORES], fp32)
            nc.sync.dma_start(out=ag_in[:], in_=nd_t[:])
            nc.gpsimd.collective_compute(
                "AllGather", Alu.bypass,
                replica_groups=[[0, 1, 2, 3, 4, 5, 6, 7]],
                ins=[ag_in.opt()], outs=[ag_out.opt()],
            )
            ag = postp.tile([1, 2 * N_CORES], fp32, tag="ag")
            nc.sync.dma_start(out=ag[:], in_=ag_out[:])
            if debug:
                nc.sync.dma_start(out=dbg_ag[:], in_=ag[:])
            # per-demog: intra_d = (num_2d + num_2d+1) / max(den_2d+den_2d+1, 1)
            num4 = postp.tile([1, ND], fp32, tag="num4")
            nc.vector.tensor_tensor(out=num4[:], in0=ag[:1, 0::4],
                                    in1=ag[:1, 2::4], op=Alu.add)
            den4 = postp.tile([1, ND], fp32, tag="den4")
            nc.vector.tensor_tensor(out=den4[:], in0=ag[:1, 1::4],
                                    in1=ag[:1, 3::4], op=Alu.add)
            nc.vector.tensor_scalar_max(den4[:], den4[:], 1.0)
            nc.vector.reciprocal(den4[:], den4[:])
            i4 = postp.tile([1, ND], fp32, tag="i4")
            nc.vector.tensor_tensor(out=i4[:], in0=num4[:], in1=den4[:],
                                    op=Alu.mult)
            mu = postp.tile([1, 1], fp32, tag="mu")
            nc.vector.tensor_reduce(out=mu[:], in_=i4[:],
                                    axis=mybir.AxisListType.X, op=Alu.add)
            nc.vector.tensor_scalar_mul(mu[:], mu[:], 1.0 / ND)
            dev = postp.tile([1, ND], fp32, tag="dev")
            nc.vector.tensor_scalar(out=dev[:], in0=i4[:], scalar1=mu[:1, :1],
                                    scalar2=None, op0=Alu.subtract)
            lo = postp.tile([1, 1], fp32, tag="lo")
            nc.vector.tensor_reduce(out=lo[:], in_=dev[:],
                                    axis=mybir.AxisListType.X, op=Alu.add,
                                    apply_absolute_value=True)
            nc.vector.tensor_scalar_mul(lo[:], lo[:], 1.0 / ND)
            nc.sync.dma_start(out=loss[:], in_=lo[:])

    nc.compile()
    return nc


def _shard(feats, labels, demog):
    """Partition rows by (demog, label-half): demog d, half h -> core 2d+h.

    Each core's 256 (demog, label) groups are disjoint from every other
    core's; within a shard, label % 256 is a bijection onto [0, 256).
    """
    half = (labels >= NL).astype(np.int32)
    shard_id = demog * 2 + half
    shards = [np.flatnonzero(shard_id == s) for s in range(N_CORES)]
    S = max(P, -(-max(len(s) for s in shards) // P) * P)
    in_maps = []
    for s in shards:
        f = np.zeros((S, D), np.float32)
        f[:len(s)] = feats[s]
        lab = np.full(S, NL, np.float32)  # pad label NL never matches iota
        lab[:len(s)] = labels[s] % NL
        lt = np.ascontiguousarray(lab.reshape(S // P, P).T)
        in_maps.append({"feats": f, "labels_t": lt})
    return S, in_maps


def kernel(feats, labels, demog_labels, _results_out=None):
    feats = np.ascontiguousarray(np.asarray(feats), dtype=np.float32)
    labels = np.asarray(labels).astype(np.int32)
    demog = np.asarray(demog_labels).astype(np.int32)
    assert feats.ndim == 2 and feats.shape[1] == D

    S, in_maps = _shard(feats, labels, demog)
    nc = _cache.get(S)
    if nc is None:
        nc = _cache.setdefault(S, _build(S))
    res = None
    last_exc = None
    for attempt in range(3):
        try:
            res = bass_utils.run_bass_kernel_spmd(
                nc, in_maps, core_ids=list(range(N_CORES)))
            break
        except Exception as e:  # transient axon worker hangups
            last_exc = e
            import time
            time.sleep(10)
    if res is None:
        raise last_exc
    if _results_out is not None:
        _results_out.append(res)
    return np.float32(res.results[0]["loss"].reshape(()))
